# revision 1
# baseline (speedup 1.0000x reference)
"""AlignBlock Trainium2 kernel — 8-core SPMD, no collectives.

Sharding: 8 cores = 2 batch x 4 time-chunks of 100 steps. Each core gets
halo-included input slices (delay-1 = 99 halo on the reference side, 4 on the
mic side for the causal conv), so cores are fully independent.

Device algorithm per core (all heavy compute on TensorEngine, bf16):
  The reference's  conv2d(QK^T sliding-window scores)  is folded into the
  score matmul via an exact rank-5 SVD of the 5x3 conv kernel expressed in
  "skewed" coordinates (query-time x, ref-time j):

      Ck[x, j] = sum_{h,r,f} Qf[h,r][f, x] * Kf[h,r][f, j]

  where Qf/Kf are the projections pre-convolved with the SVD row/col factors.
  One PSUM accumulation over 101 k-chunks of 128 computes scores+conv at once.
  The conv's zero-padding at the delay edges (d = -1, d = 100) is restored by
  an exact correction baked into the additive softmax mask's two edge
  diagonals. Softmax (mask-add, -max, exp with fused row-sum) runs on
  DVE/ACT; the attention weights are transposed by the TensorEngine and
  applied to raw x_ref windows with a second matmul group; the 1/sum
  normalization rides the PSUM->SBUF output copy.
"""

import numpy as np
import ml_dtypes

B, C, H, T, F, DELAY = 2, 16, 16, 400, 161, 100
TL = 100            # output timesteps per core
QT = TL + 4         # mic slice length (causal conv halo)
KT = TL + 103       # ref slice length (window + conv halos)
RANK = 5
NF8 = 4                       # lowest-sigma ranks stored/multiplied in fp8
RBF = RANK - NF8              # bf16 ranks
BF_ROWS = H * RBF * F         # 5152
F8_ROWS = H * NF8 * F         # 7728
NCH_BF = (BF_ROWS + 127) // 128   # 41
NCH_F8 = (F8_ROWS + 127) // 128   # 61
NOC = 7                       # output column chunks (7 x 368 = 16*161)
OCW = (C * F) // NOC          # 368
GROUPS_BF = [11, 10]          # bf16 k-chunk DMA groups (21)
GROUPS_F8 = [9, 18, 18, 18, 18]  # fp8 k-chunk DMA groups (81)

BF16 = ml_dtypes.bfloat16
FP8 = ml_dtypes.float8_e4m3

_CACHE = {}


def _build_raw():
    if "ncr" in _CACHE:
        return _CACHE["ncr"]
    import concourse.bass as bass
    from concourse import bacc, mybir

    dt = mybir.dt
    nc = bacc.Bacc("TRN2", target_bir_lowering=False, debug=False, num_devices=8)

    f8_d = nc.dram_tensor("factf8", [128, NCH_F8, TL + KT], dt.float8e4, kind="ExternalInput").ap()
    fb_d = nc.dram_tensor("factbf", [128, NCH_BF, TL + KT], dt.bfloat16, kind="ExternalInput").ap()
    xr_d = nc.dram_tensor("xr", [KT, C * F], dt.bfloat16, kind="ExternalInput").ap()
    mask_d = nc.dram_tensor("mask", [TL, KT], dt.float32, kind="ExternalInput").ap()
    id_d = nc.dram_tensor("ident", [128, 128], dt.bfloat16, kind="ExternalInput").ap()
    out_d = nc.dram_tensor("out", [TL, C * F], dt.float32, kind="ExternalOutput").ap()

    # static SBUF
    g8 = [nc.alloc_sbuf_tensor(f"g8_{i}", [128, gn, TL + KT], dt.float8e4).ap()
          for i, gn in enumerate(GROUPS_F8)]
    gb = [nc.alloc_sbuf_tensor(f"gb_{i}", [128, gn, TL + KT], dt.bfloat16).ap()
          for i, gn in enumerate(GROUPS_BF)]
    xr01 = nc.alloc_sbuf_tensor("xr01", [128, 2, C * F], dt.bfloat16).ap()
    mask = nc.alloc_sbuf_tensor("mask_sb", [TL, KT], dt.float32).ap()
    ident = nc.alloc_sbuf_tensor("ident_sb", [128, 128], dt.bfloat16).ap()
    ckm = nc.alloc_sbuf_tensor("ckm", [TL, KT], dt.float32).ap()
    eb = nc.alloc_sbuf_tensor("eb", [TL, KT], dt.bfloat16).ap()
    nmx = nc.alloc_sbuf_tensor("nmx", [TL, 1], dt.float32).ap()
    ssum = nc.alloc_sbuf_tensor("ssum", [TL, 1], dt.float32).ap()
    rinv = nc.alloc_sbuf_tensor("rinv", [TL, 1], dt.float32).ap()
    a0 = nc.alloc_sbuf_tensor("a0", [128, TL], dt.bfloat16).ap()
    a1 = nc.alloc_sbuf_tensor("a1", [KT - 128, TL], dt.bfloat16).ap()
    ob = nc.alloc_sbuf_tensor("ob", [TL, C * F], dt.float32).ap()
    warm = nc.alloc_sbuf_tensor("warm", [1, 1], dt.float32).ap()

    ck = nc.alloc_psum_tensor("ck", [TL, KT], dt.float32).ap()
    tp0 = nc.alloc_psum_tensor("tp0", [128, TL], dt.bfloat16).ap()
    tp1 = nc.alloc_psum_tensor("tp1", [128, TL], dt.bfloat16).ap()
    po = [nc.alloc_psum_tensor(f"po{i}", [TL, OCW], dt.float32).ap() for i in range(4)]

    NGRP = len(GROUPS_F8) + len(GROUPS_BF)
    AF = mybir.ActivationFunctionType

    with (
        nc.Block(no_gpsimd_drain=True) as block,
        nc.semaphore("dg0") as dg0,
        nc.semaphore("dg1") as dg1,
        nc.semaphore("dg2") as dg2,
        nc.semaphore("dg3") as dg3,
        nc.semaphore("dg4") as dg4,
        nc.semaphore("dg5") as dg5,
        nc.semaphore("dg6") as dg6,
        nc.semaphore("csem") as csem,
        nc.semaphore("tsem") as tsem,
        nc.semaphore("vsem") as vsem,
        nc.semaphore("esem") as esem,
        nc.semaphore("rsem") as rsem,
        nc.semaphore("tpsem") as tpsem,
        nc.semaphore("asem") as asem,
        nc.semaphore("s6sem") as s6sem,
        nc.semaphore("cpv") as cpv,
        nc.semaphore("cps") as cps,
        nc.semaphore("odsem") as odsem,
    ):
        @block.sync
        def _(sync):
            # even factor groups on the sync HWDGE ring (odd ones ride the
            # scalar ring) — halves per-ring issue serialization
            dgs = [dg0, dg1, dg2, dg3, dg4, dg5, dg6]
            for i, _gn in enumerate(GROUPS_F8):
                if i % 2 == 0:
                    sync.dma_start(out=g8[i][:], in_=f8_d[:, sum(GROUPS_F8[:i]):sum(GROUPS_F8[:i + 1]), :]).then_inc(dgs[i], 16)
            nb = len(GROUPS_F8)
            for i, _gn in enumerate(GROUPS_BF):
                if (nb + i) % 2 == 0:
                    sync.dma_start(out=gb[i][:], in_=fb_d[:, sum(GROUPS_BF[:i]):sum(GROUPS_BF[:i + 1]), :]).then_inc(dgs[nb + i], 16)
            pieces = [(0, 2, 1, 1), (2, 4, 2, 2), (4, 6, 3, 3), (6, 7, 4, 3)]
            for lo, hi, vneed, sneed in pieces:
                sync.wait_ge(cpv, vneed)
                sync.wait_ge(cps, sneed)
                sync.dma_start(out=out_d[:, lo * OCW:hi * OCW],
                               in_=ob[:, lo * OCW:hi * OCW]).then_inc(odsem, 16)
            sync.wait_ge(odsem, 64)

        @block.scalar
        def _(scalar):
            # pre-load the exp table first; const loads ride the scalar HWDGE
            # ring, with the big xr load deferred past the factor-DMA ramp
            scalar.activation(warm[:], warm[:], AF.Exp)
            dgs = [dg0, dg1, dg2, dg3, dg4, dg5, dg6]
            nb = len(GROUPS_F8)
            for i, _gn in enumerate(GROUPS_F8):
                if i % 2 == 1:
                    scalar.dma_start(out=g8[i][:], in_=f8_d[:, sum(GROUPS_F8[:i]):sum(GROUPS_F8[:i + 1]), :]).then_inc(dgs[i], 16)
            for i, _gn in enumerate(GROUPS_BF):
                if (nb + i) % 2 == 1:
                    scalar.dma_start(out=gb[i][:], in_=fb_d[:, sum(GROUPS_BF[:i]):sum(GROUPS_BF[:i + 1]), :]).then_inc(dgs[nb + i], 16)
            scalar.dma_start(out=mask[:], in_=mask_d[:]).then_inc(csem, 16)
            scalar.dma_start(out=ident[:], in_=id_d[:]).then_inc(csem, 16)
            scalar.dma_start(out=xr01[:, 0, :], in_=xr_d[0:128, :]).then_inc(csem, 16)
            scalar.dma_start(out=xr01[0:KT - 128, 1, :], in_=xr_d[128:KT, :]).then_inc(csem, 16)
            # softmax exp
            scalar.wait_ge(vsem, 1)
            scalar.activation(eb[:], ckm[:], AF.Exp, bias=nmx[:], scale=1.0,
                              accum_out=ssum[:]).then_inc(esem, 1)
            # transpose copy (upper part)
            scalar.wait_ge(tpsem, 2)
            scalar.activation(warm[:], warm[:], AF.Exp)
            scalar.copy(a1[:], tp1[0:KT - 128, :]).then_inc(asem, 1)
            # epilogue: odd output chunks
            scalar.wait_ge(rsem, 1)
            for n in (1, 3, 5):
                scalar.wait_ge(s6sem, n + 1)
                scalar.activation(ob[:, n * OCW:(n + 1) * OCW], po[n % 4][:],
                                  AF.Copy, bias=0.0, scale=rinv[:]).then_inc(cps, 1)

        @block.tensor
        def _(tensor):
            cdone = 0
            nch_total = NCH_F8 + NCH_BF
            bufs = list(zip(GROUPS_F8, g8)) + list(zip(GROUPS_BF, gb))
            dgs = [dg0, dg1, dg2, dg3, dg4, dg5, dg6]
            for g, (gn, buf) in enumerate(bufs):
                tensor.wait_ge(dgs[g], 16)
                for i in range(gn):
                    last = cdone + i == nch_total - 1
                    tensor.matmul(ck[:], buf[:, i, 0:TL], buf[:, i, TL:TL + KT],
                                  start=(cdone + i == 0), stop=last)
                cdone += gn
            # drain fence: a >=128-column matmul issued after the stop-matmul
            # retires only after the previous matmul's systolic drain has
            # fully landed in PSUM, so its then_inc safely publishes ck.
            lastbuf = bufs[-1][1]
            tensor.matmul(po[0][0:TL, 0:128], lastbuf[:, 0, 0:TL],
                          lastbuf[:, 0, TL:TL + 128],
                          start=True, stop=True).then_inc(tsem, 1)
            # transposes of attention weights
            tensor.wait_ge(esem, 1)
            tensor.wait_ge(csem, 64)
            tensor.transpose(tp0[:], eb[:, 0:128], ident[0:TL, 0:TL]).then_inc(tpsem, 1)
            tensor.transpose(tp1[0:KT - 128, :], eb[:, 128:KT], ident[0:TL, 0:TL]).then_inc(tpsem, 1)
            # stage 6
            tensor.wait_ge(asem, 2)
            tensor.wait_ge(csem, 64)
            for n in range(NOC):
                if n >= 4:
                    m = n - 4  # buffer po[m % 4] must be drained
                    if m % 2 == 0:
                        tensor.wait_ge(cpv, m // 2 + 1)
                    else:
                        tensor.wait_ge(cps, m // 2 + 1)
                mm1 = tensor.matmul(po[n % 4][:], a0[:, :],
                                    xr01[:, 0, n * OCW:(n + 1) * OCW],
                                    start=True, stop=False)
                if n >= 1:
                    # publishes pair n-1 (drain-fenced by this 368-col stream)
                    mm1.then_inc(s6sem, 1)
                tensor.matmul(po[n % 4][:], a1[:, :],
                              xr01[0:KT - 128, 1, n * OCW:(n + 1) * OCW],
                              start=False, stop=True)
            # fence for the last two pairs: ck's bank is long consumed
            tensor.matmul(ck[0:TL, 0:128], a0[:, 0:TL], xr01[:, 0, 0:128],
                          start=True, stop=True).then_inc(s6sem, 2)

        @block.vector
        def _(vector):
            vector.memset(warm[:], 0.0)
            vector.wait_ge(tsem, 1)
            vector.wait_ge(csem, 64)
            vector.tensor_add(ckm[:], ck[:], mask[:])
            vector.tensor_reduce(nmx[:], ckm[:], axis=mybir.AxisListType.X,
                                 op=mybir.AluOpType.max, negate=True).then_inc(vsem, 1)
            vector.wait_ge(esem, 1)
            vector.reciprocal(rinv[:], ssum[:]).then_inc(rsem, 1)
            vector.wait_ge(tpsem, 1)
            vector.memset(warm[:], 0.0)
            vector.memset(warm[:], 0.0)
            vector.tensor_copy(a0[:], tp0[:]).then_inc(asem, 1)
            # epilogue: even output chunks
            for n in (0, 2, 4, 6):
                vector.wait_ge(s6sem, n + 1)
                vector.tensor_scalar_mul(ob[:, n * OCW:(n + 1) * OCW], po[n % 4][:],
                                         rinv[:]).then_inc(cpv, 1)

    nc.compile()
    _CACHE["ncr"] = nc
    return nc


def _host_prep(x_mic, x_ref, w_mic, b_mic, w_ref, b_ref, w_conv, b_conv):
    """Build the 8 per-core input maps (layout prep + tiny projections)."""
    wc = w_conv[0]                       # (H, 5, 3)
    # skewed kernel G[h, p, t], t = p + kw in [0, 7)
    G = np.zeros((H, 5, 7), dtype=np.float64)
    for p in range(5):
        for kw in range(3):
            G[:, p, p + kw] = wc[:, p, kw]
    Us = np.zeros((H, 5, RANK)); Vs = np.zeros((H, RANK, 7))
    for h in range(H):
        u, s, vt = np.linalg.svd(G[h])
        Us[h] = u[:, :RANK] * s[:RANK]
        Vs[h] = vt[:RANK]

    ident = np.eye(128, dtype=BF16)
    in_maps = []
    core_meta = []
    for b in range(B):
        for tc_ in range(T // TL):
            t0 = tc_ * TL
            qi = np.arange(t0 - 4, t0 + TL)
            ji = np.arange(t0 - 103, t0 + TL)
            mv = (qi >= 0).astype(np.float32)
            jv = (ji >= 0).astype(np.float32)
            xm = x_mic[b][:, np.clip(qi, 0, None), :] * mv[None, :, None]
            xr = x_ref[b][:, np.clip(ji, 0, None), :] * jv[None, :, None]
            # projections (h, t, f); bias masked to keep padded region zero
            Qh = np.einsum('hc,cif->hif', w_mic, xm) + b_mic[:, None, None] * mv[None, :, None]
            Kh = np.einsum('hc,cjf->hjf', w_ref, xr) + b_ref[:, None, None] * jv[None, :, None]
            # factors
            Qf = np.zeros((H, RANK, F, TL), dtype=np.float32)
            for p in range(5):
                Qf += Us[:, p, :, None, None].astype(np.float32) \
                    * Qh[:, None, p:p + TL, :].transpose(0, 1, 3, 2)
            Kp = np.pad(Kh, ((0, 0), (5, 1), (0, 0)))
            Kf = np.zeros((H, RANK, F, KT), dtype=np.float32)
            for t in range(7):
                Kf += Vs[:, :, t, None, None].astype(np.float32) \
                    * Kp[:, None, t:t + KT, :].transpose(0, 1, 3, 2)
            # r-major rows (r, h, f); ranks [0,RBF) -> bf16, rest -> fp8
            Qr = Qf.transpose(1, 0, 2, 3).reshape(RANK, H * F, TL)
            Kr = Kf.transpose(1, 0, 2, 3).reshape(RANK, H * F, KT)
            def pack(qpart, kpart, nch, npdtype):
                rows = qpart.shape[0] * qpart.shape[1]
                fa = np.zeros((nch * 128, TL + KT), dtype=npdtype)
                fa[:rows, :TL] = qpart.reshape(rows, TL)
                fa[:rows, TL:] = kpart.reshape(rows, KT)
                return fa.reshape(nch, 128, TL + KT).transpose(1, 0, 2).copy()
            fbf = pack(Qr[:RBF], Kr[:RBF], NCH_BF, BF16)
            ff8 = pack(Qr[RBF:], Kr[RBF:], NCH_F8, FP8)
            # additive mask: -30000 outside band, exact edge-leak correction
            x_idx = np.arange(TL)[:, None]
            j_idx = np.arange(KT)[None, :]
            band = (j_idx >= x_idx + 4) & (j_idx <= x_idx + 103)
            Kp3 = np.pad(Kh, ((0, 0), (1, 1), (0, 0)))
            vd_m1 = np.einsum('hif,hif->hi', Qh, Kp3[:, 0:QT, :])
            vd_p100 = np.einsum('hif,hif->hi', Qh, Kp3[:, 101:101 + QT, :])
            xv = np.arange(TL)
            Gd0 = G[:, np.arange(5), np.arange(5)]          # kw=0 tap weights
            Gd2 = G[:, np.arange(5), np.arange(5) + 2]      # kw=2 tap weights
            leak0 = np.einsum('hk,hxk->x', Gd0,
                              np.stack([vd_m1[:, xv + k] for k in range(5)], -1))
            leak99 = np.einsum('hk,hxk->x', Gd2,
                               np.stack([vd_p100[:, xv + k] for k in range(5)], -1))
            mask = np.where(band, 0.0, -30000.0).astype(np.float32)
            mask[xv, xv + 4] -= leak0.astype(np.float32)
            mask[xv, xv + 103] -= leak99.astype(np.float32)
            # raw x_ref for the value matmul: [j, (c, f)]
            xrb = np.ascontiguousarray(
                xr.transpose(1, 0, 2).reshape(KT, C * F).astype(BF16))
            in_maps.append({
                "factbf": fbf, "factf8": ff8, "xr": xrb, "mask": mask,
                "ident": ident,
            })
            core_meta.append((b, t0))
    return in_maps, core_meta


def kernel(**inputs):
    x_mic = np.asarray(inputs["x_mic"], dtype=np.float32)
    x_ref = np.asarray(inputs["x_ref"], dtype=np.float32)
    w_mic = np.asarray(inputs["w_mic"], dtype=np.float32)
    b_mic = np.asarray(inputs["b_mic"], dtype=np.float32)
    w_ref = np.asarray(inputs["w_ref"], dtype=np.float32)
    b_ref = np.asarray(inputs["b_ref"], dtype=np.float32)
    w_conv = np.asarray(inputs["w_conv"], dtype=np.float32)
    b_conv = np.asarray(inputs["b_conv"], dtype=np.float32)
    delay = int(inputs["delay"])
    assert delay == DELAY, f"kernel hardcodes delay={DELAY}, got {delay}"

    in_maps, core_meta = _host_prep(
        x_mic, x_ref, w_mic, b_mic, w_ref, b_ref, w_conv, b_conv
    )
    nc = _build_raw()
    from concourse.bass_utils import run_bass_kernel_spmd

    res = run_bass_kernel_spmd(nc, in_maps, core_ids=list(range(8)))
    out = np.zeros((B, C, T, F), dtype=np.float32)
    for (b, t0), r in zip(core_meta, res.results):
        o = np.asarray(r["out"], dtype=np.float32).reshape(TL, C, F)
        out[b, :, t0:t0 + TL, :] = o.transpose(1, 0, 2)
    return out


if __name__ == "__main__":
    z = np.load("/tmp/inputs.npz")
    ins = {k: z[k] for k in z.files}
    out = kernel(**ins)
    ref = np.load("/tmp/ref.npy")
    rel = np.abs(out - ref).max() / np.abs(ref).max()
    print("Relative error:", rel)



# revision 12
# speedup vs baseline: 1.0611x; 1.0611x over previous
"""AlignBlock Trainium2 kernel — 8-core SPMD, no collectives.

Sharding: 8 cores = 2 batch x 4 time-chunks of 100 steps, fully independent
(halo-included input slices).

Device algorithm per core (v2 — "shifted K-conv variants"):
  The 5x3 conv over (t, d) of the QK^T scores is folded EXACTLY into the
  score matmul by pre-convolving the K projection with the 3 d-taps for each
  of the 5 time taps i:

      KG_i[k, y] = sum_j' wc[h,i,j'] * Kh[k, y + j' - 1]          (k = (h,f))
      Ck[x, j]   = sum_i sum_k Q[k, x+i-4] * KG_i[k, j+i-4]

  The Q-side time shifts are free SBUF column offsets of one shared Q buffer
  (104 cols); the K-side shifts are baked into each variant's column layout.
  This ships 5x fp8 K-variants + 1x bf16 Q instead of rank-5 SVD factors of
  BOTH sides: 3.4 MB instead of 4.8 MB of score-feeding DMA. KG is scaled by
  64 (inverse folded into bf16 Q) to clear fp8e4m3's subnormal floor.

  The additive softmax mask (band + exact d-edge leak corrections + conv
  bias) is folded into the same PSUM accumulation as an identity-weighted
  bf16 matmul, so softmax is just exp() on ACT straight out of PSUM (logits
  bounded, no max pass). Attention weights are transposed on the PE and
  applied to raw bf16 x_ref windows in two stationary-weight rounds over 6
  output column chunks; 1/rowsum rides the PSUM->SBUF output copies (bf16).
"""

import numpy as np
import ml_dtypes

B, C, H, T, F, DELAY = 2, 16, 16, 400, 161, 100
TL = 100            # output timesteps per core
QT = TL + 4         # mic-side cols (causal conv halo)
KT = TL + 103       # ref-side cols (window + conv halos)
NV = 5              # conv time taps = K variants
NCH = 21            # 128-row chunks per variant (H*F = 2576 rows)
TOTCH = NV * NCH    # 105
KSCALE = 64.0       # fp8 pre-scale on KG, inverse folded into Q
NEG = -60.0         # out-of-band additive mask
VB = [0, 432, 864, 1296, 1728, 2160, 2576]   # value/output column chunks
NPO = 5             # PSUM banks for value chunks (chunk 5 reuses bank 0)

BF16 = ml_dtypes.bfloat16
FP8 = ml_dtypes.float8_e4m3

_CACHE = {}


def _build_raw():
    if "ncr" in _CACHE:
        return _CACHE["ncr"]
    import concourse.bass as bass
    from concourse import bacc, mybir

    dt = mybir.dt
    nc = bacc.Bacc("TRN2", target_bir_lowering=False, debug=False, num_devices=8)

    q_d = nc.dram_tensor("qf", [128, NCH, QT], dt.bfloat16, kind="ExternalInput").ap()
    kg_d = nc.dram_tensor("kg", [128, TOTCH, KT], dt.float8e4, kind="ExternalInput").ap()
    mask_d = nc.dram_tensor("mask", [128, KT], dt.bfloat16, kind="ExternalInput").ap()
    id_d = nc.dram_tensor("ident", [128, 128], dt.bfloat16, kind="ExternalInput").ap()
    xr_d = nc.dram_tensor("xr", [KT, C * F], dt.bfloat16, kind="ExternalInput").ap()
    out_d = nc.dram_tensor("out", [TL, C * F], dt.bfloat16, kind="ExternalOutput").ap()

    # static SBUF
    qb = nc.alloc_sbuf_tensor("qb", [128, NCH, QT], dt.bfloat16).ap()
    kgb = nc.alloc_sbuf_tensor("kgb", [128, TOTCH, KT], dt.float8e4).ap()
    maskb = nc.alloc_sbuf_tensor("maskb", [128, KT], dt.bfloat16).ap()
    identb = nc.alloc_sbuf_tensor("identb", [128, 128], dt.bfloat16).ap()
    xr01 = nc.alloc_sbuf_tensor("xr01", [128, 2, C * F], dt.bfloat16).ap()
    eb = nc.alloc_sbuf_tensor("eb", [TL, KT], dt.bfloat16).ap()
    ssum = nc.alloc_sbuf_tensor("ssum", [TL, 1], dt.float32).ap()
    rinv = nc.alloc_sbuf_tensor("rinv", [TL, 1], dt.float32).ap()
    a0 = nc.alloc_sbuf_tensor("a0", [128, TL], dt.bfloat16).ap()
    a1 = nc.alloc_sbuf_tensor("a1", [KT - 128, TL], dt.bfloat16).ap()
    ob = nc.alloc_sbuf_tensor("ob", [TL, C * F], dt.bfloat16).ap()
    warm = nc.alloc_sbuf_tensor("warm", [1, 2], dt.float32).ap()

    ck = nc.alloc_psum_tensor("ck", [TL, KT], dt.float32).ap()
    tp0 = nc.alloc_psum_tensor("tp0", [128, TL], dt.bfloat16).ap()
    tp1 = nc.alloc_psum_tensor("tp1", [128, TL], dt.bfloat16).ap()
    po = [nc.alloc_psum_tensor(f"po{i}", [TL, 432], dt.float32).ap()
          for i in range(NPO)]

    AF = mybir.ActivationFunctionType
    # KG DMA groups: (lo, hi) chunk ranges; ring A = sync, ring B = scalar
    GA = [(0, 15), (30, 45), (60, 75), (90, 105)]
    GB = [(15, 30), (45, 60), (75, 90)]
    XSPLIT = 1296   # xr / out column split between the two rings / pieces

    from contextlib import ExitStack

    with ExitStack() as stack:
        block = stack.enter_context(nc.Block(no_gpsimd_drain=True))
        names = ["smC", "smQa", "smQb", "sg0", "sg1", "sg2", "sg3", "sg4",
                 "sg5", "sg6", "sxA", "sxB", "tsem", "esem", "tpsem", "asem",
                 "rsem", "s6a", "s6b", "cqv", "cqs", "odsem", "wsem"]
        sem = {n: stack.enter_context(nc.semaphore(n)) for n in names}
        (smC, smQa, smQb, sg0, sg1, sg2, sg3, sg4, sg5, sg6, sxA, sxB, tsem,
         esem, tpsem, asem, rsem, s6a, s6b, cqv, cqs, odsem, wsem) = (
            sem[n] for n in names)
        sgrp = {0: sg0, 15: sg1, 30: sg2, 45: sg3, 60: sg4, 75: sg5, 90: sg6}

        @block.sync
        def _(sync):
            sync.dma_start(out=identb[:], in_=id_d[:]).then_inc(smC, 16)
            sync.dma_start(out=maskb[:], in_=mask_d[:]).then_inc(smC, 16)
            for (lo, hi) in GA:
                sync.dma_start(out=kgb[:, lo:hi, :],
                               in_=kg_d[:, lo:hi, :]).then_inc(sgrp[lo], 16)
            sync.dma_start(out=xr01[:, 0, 0:XSPLIT],
                           in_=xr_d[0:128, 0:XSPLIT]).then_inc(sxA, 16)
            sync.dma_start(out=xr01[0:KT - 128, 1, 0:XSPLIT],
                           in_=xr_d[128:KT, 0:XSPLIT]).then_inc(sxA, 16)
            sync.wait_ge(cqv, 2)
            sync.wait_ge(cqs, 1)
            sync.dma_start(out=out_d[:, 0:XSPLIT],
                           in_=ob[:, 0:XSPLIT]).then_inc(odsem, 16)
            sync.wait_ge(cqv, 3)
            sync.wait_ge(cqs, 3)
            sync.dma_start(out=out_d[:, XSPLIT:],
                           in_=ob[:, XSPLIT:]).then_inc(odsem, 16)
            sync.wait_ge(odsem, 32)

        @block.scalar
        def _(scalar):
            # pre-load the exp + copy activation tables; rides the scalar queue
            # while the DMA ramp is still cold
            scalar.wait_ge(wsem, 1)
            scalar.activation(warm[:, 0:1], warm[:, 0:1], AF.Exp)
            scalar.copy(warm[:, 1:2], warm[:, 1:2])
            scalar.dma_start(out=qb[:, 0:11, :], in_=q_d[:, 0:11, :]).then_inc(smQa, 16)
            scalar.dma_start(out=qb[:, 11:NCH, :], in_=q_d[:, 11:NCH, :]).then_inc(smQb, 16)
            for (lo, hi) in GB:
                scalar.dma_start(out=kgb[:, lo:hi, :],
                                 in_=kg_d[:, lo:hi, :]).then_inc(sgrp[lo], 16)
            scalar.dma_start(out=xr01[:, 0, XSPLIT:],
                             in_=xr_d[0:128, XSPLIT:]).then_inc(sxB, 16)
            scalar.dma_start(out=xr01[0:KT - 128, 1, XSPLIT:],
                             in_=xr_d[128:KT, XSPLIT:]).then_inc(sxB, 16)
            # softmax exp straight off PSUM (mask already folded in)
            scalar.wait_ge(tsem, 1)
            scalar.activation(eb[:], ck[:], AF.Exp, bias=0.0, scale=1.0).then_inc(esem, 1)
            # attention-weight transpose copy (lower part)
            scalar.wait_ge(tpsem, 1)
            scalar.copy(a1[:], tp1[0:KT - 128, :]).then_inc(asem, 1)
            # output copies: odd chunks (1, 3, 5), 1/rowsum folded into scale
            scalar.wait_ge(s6a, 1)
            scalar.wait_ge(rsem, 2)
            scalar.activation(ob[:, VB[1]:VB[2]], po[1][:],
                              AF.Copy, bias=0.0, scale=rinv[:]).then_inc(cqs, 1)
            scalar.wait_ge(s6b, 1)
            scalar.activation(ob[:, VB[3]:VB[4]], po[3][:],
                              AF.Copy, bias=0.0, scale=rinv[:]).then_inc(cqs, 1)
            scalar.activation(ob[:, VB[5]:VB[6]], po[0][:, 0:VB[6] - VB[5]],
                              AF.Copy, bias=0.0, scale=rinv[:]).then_inc(cqs, 1)

        @block.tensor
        def _(tensor):
            # mask + leak corrections + conv bias enter the accumulation first
            tensor.wait_ge(smC, 32)
            tensor.matmul(ck[:], identb[0:TL, 0:TL], maskb[0:TL, :],
                          start=True, stop=False)
            tensor.wait_ge(smQa, 16)
            for cc in range(TOTCH):
                if cc in sgrp:
                    tensor.wait_ge(sgrp[cc], 16)
                if cc == 11:
                    tensor.wait_ge(smQb, 16)
                i, c = cc // NCH, cc % NCH
                tensor.matmul(ck[:], qb[:, c, i:i + TL], kgb[:, cc, :],
                              start=False, stop=(cc == TOTCH - 1))
            # drain fence: a >=128-column matmul issued after the stop-matmul
            # retires only after the accumulation has fully landed in PSUM
            tensor.matmul(po[0][:, 0:128], kgb[:, 0, 0:TL], kgb[:, 0, 0:128],
                          start=True, stop=True).then_inc(tsem, 1)
            # transposes of attention weights + drain fence
            tensor.wait_ge(esem, 1)
            tensor.transpose(tp0[:], eb[:, 0:128], identb[0:TL, 0:TL])
            tensor.transpose(tp1[0:KT - 128, :], eb[:, 128:KT], identb[0:TL, 0:TL])
            tensor.matmul(ck[:, 0:128], identb[:, 0:TL], identb[:, 0:128],
                          start=True, stop=True).then_inc(tpsem, 1)
            # value matmuls: round A = chunks 0-2, round B = chunks 3-5;
            # two stationary loads per round (a0 then a1)
            tensor.wait_ge(asem, 2)
            tensor.wait_ge(sxA, 32)
            for n in (0, 1, 2):
                tensor.matmul(po[n][:, 0:VB[n + 1] - VB[n]], a0[:, :],
                              xr01[:, 0, VB[n]:VB[n + 1]], start=True, stop=False)
            for n in (0, 1, 2):
                tensor.matmul(po[n][:, 0:VB[n + 1] - VB[n]], a1[:, :],
                              xr01[0:KT - 128, 1, VB[n]:VB[n + 1]],
                              start=False, stop=True)
            # publish round A (drain-fenced junk matmul), decoupled from sxB
            tensor.matmul(ck[:, 0:128], identb[:, 0:TL], identb[:, 0:128],
                          start=True, stop=True).then_inc(s6a, 1)
            tensor.wait_ge(sxB, 32)
            for n in (3, 4, 5):
                if n == 5:
                    tensor.wait_ge(cqv, 1)   # chunk 0's copy frees po[0]
                tensor.matmul(po[n % NPO][:, 0:VB[n + 1] - VB[n]], a0[:, :],
                              xr01[:, 0, VB[n]:VB[n + 1]], start=True, stop=False)
            for n in (3, 4, 5):
                tensor.matmul(po[n % NPO][:, 0:VB[n + 1] - VB[n]], a1[:, :],
                              xr01[0:KT - 128, 1, VB[n]:VB[n + 1]],
                              start=False, stop=True)
            tensor.matmul(ck[:, 0:128], identb[:, 0:TL], identb[:, 0:128],
                          start=True, stop=True).then_inc(s6b, 1)

        @block.vector
        def _(vector):
            vector.memset(warm[:], 0.0).then_inc(wsem, 1)
            # attention-weight transpose copy (upper part)
            vector.wait_ge(tpsem, 1)
            vector.tensor_copy(a0[:], tp0[:]).then_inc(asem, 1)
            # row sums + reciprocal (off the transpose critical path)
            vector.tensor_reduce(ssum[:], eb[:], axis=mybir.AxisListType.X,
                                 op=mybir.AluOpType.add).then_inc(rsem, 1)
            vector.wait_ge(rsem, 1)
            vector.reciprocal(rinv[:], ssum[:]).then_inc(rsem, 1)
            # output copies: even chunks (0, 2, 4)
            vector.wait_ge(rsem, 2)
            vector.wait_ge(s6a, 1)
            vector.tensor_scalar_mul(ob[:, VB[0]:VB[1]], po[0][:],
                                     rinv[:]).then_inc(cqv, 1)
            vector.tensor_scalar_mul(ob[:, VB[2]:VB[3]], po[2][:],
                                     rinv[:]).then_inc(cqv, 1)
            vector.wait_ge(s6b, 1)
            vector.tensor_scalar_mul(ob[:, VB[4]:VB[5]], po[4][:],
                                     rinv[:]).then_inc(cqv, 1)

    nc.compile()
    _CACHE["ncr"] = nc
    return nc


def _host_prep(x_mic, x_ref, w_mic, b_mic, w_ref, b_ref, w_conv, b_conv):
    """Build the 8 per-core input maps (layout prep + tiny 1x1 projections)."""
    f32 = np.float32
    wc = w_conv[0]                                   # (H, 5, 3)
    Qh = np.einsum("hc,bctf->bhtf", w_mic, x_mic) + b_mic[None, :, None, None]
    Kh = np.einsum("hc,bctf->bhtf", w_ref, x_ref) + b_ref[None, :, None, None]
    PAD = 120
    Khp = np.pad(Kh, ((0, 0), (0, 0), (PAD, PAD), (0, 0)))
    Qhp = np.pad(Qh, ((0, 0), (0, 0), (8, 8), (0, 0)))
    xrp = np.pad(x_ref, ((0, 0), (0, 0), (PAD, PAD), (0, 0)))
    L = T + 2 * PAD
    # KGg[i][b,h,m,f] = sum_j' wc[h,i,j'] Khp[m + j'], tau(m) = m + 1 - PAD
    KGg = np.zeros((NV, B, H, L - 2, F), f32)
    for i in range(NV):
        for jp in range(3):
            KGg[i] += wc[:, i, jp][None, :, None, None] * Khp[:, :, jp:jp + L - 2, :]

    ident = np.eye(128, dtype=BF16)
    in_maps, core_meta = [], []
    for b in range(B):
        for tc in range(T // TL):
            t0 = tc * TL
            Qb = Qhp[b][:, t0 + 4:t0 + 4 + QT, :]            # x' in [-4, 100)
            qrows = Qb.transpose(0, 2, 1).reshape(H * F, QT) / KSCALE
            qp = np.zeros((NCH * 128, QT), f32)
            qp[:H * F] = qrows
            qpack = np.ascontiguousarray(
                qp.reshape(NCH, 128, QT).transpose(1, 0, 2)).astype(BF16)
            # K variants, column-shifted so all matmuls read cols [0, KT)
            kgp = np.zeros((TOTCH, 128, KT), f32)
            for i in range(NV):
                m0 = t0 - 108 + i + PAD                      # tau = t0-107+i+j2
                sl = KGg[i, b][:, m0:m0 + KT, :]
                rows = sl.transpose(0, 2, 1).reshape(H * F, KT) * KSCALE
                tmp = np.zeros((NCH * 128, KT), f32)
                tmp[:H * F] = rows
                kgp[i * NCH:(i + 1) * NCH] = tmp.reshape(NCH, 128, KT)
            kgpack = np.ascontiguousarray(kgp.transpose(1, 0, 2)).astype(FP8)
            # additive mask: band + exact d-edge leak corrections + conv bias
            x_idx = np.arange(TL)[:, None]
            j_idx = np.arange(KT)[None, :]
            band = (j_idx >= x_idx + 4) & (j_idx <= x_idx + 103)
            mask = np.where(band, 0.0, NEG).astype(f32)
            xs = np.arange(-4, TL)
            Dm1 = np.einsum("hxf,hxf->hx", Qb, Khp[b][:, t0 + xs - 100 + PAD, :])
            Dp1 = np.einsum("hxf,hxf->hx", Qb, Khp[b][:, t0 + xs + 1 + PAD, :])
            xv = np.arange(TL)
            leak0 = np.zeros(TL, f32)
            leak99 = np.zeros(TL, f32)
            for i in range(NV):
                leak0 += wc[:, i, 0] @ Dm1[:, xv + i]
                leak99 += wc[:, i, 2] @ Dp1[:, xv + i]
            mask[xv, xv + 4] -= leak0
            mask[xv, xv + 103] -= leak99
            mask += float(np.asarray(b_conv).reshape(-1)[0])
            maskp = np.zeros((128, KT), f32)
            maskp[:TL] = mask
            # raw x_ref windows for the value matmul: [j, (c, f)]
            jt = t0 - 103 + np.arange(KT)
            xrb = np.ascontiguousarray(
                xrp[b][:, jt + PAD, :].transpose(1, 0, 2).reshape(KT, C * F)
            ).astype(BF16)
            in_maps.append({
                "qf": qpack, "kg": kgpack, "mask": maskp.astype(BF16),
                "ident": ident, "xr": xrb,
            })
            core_meta.append((b, t0))
    return in_maps, core_meta


def kernel(**inputs):
    x_mic = np.asarray(inputs["x_mic"], dtype=np.float32)
    x_ref = np.asarray(inputs["x_ref"], dtype=np.float32)
    w_mic = np.asarray(inputs["w_mic"], dtype=np.float32)
    b_mic = np.asarray(inputs["b_mic"], dtype=np.float32)
    w_ref = np.asarray(inputs["w_ref"], dtype=np.float32)
    b_ref = np.asarray(inputs["b_ref"], dtype=np.float32)
    w_conv = np.asarray(inputs["w_conv"], dtype=np.float32)
    b_conv = np.asarray(inputs["b_conv"], dtype=np.float32)
    delay = int(inputs["delay"])
    assert delay == DELAY, f"kernel hardcodes delay={DELAY}, got {delay}"

    in_maps, core_meta = _host_prep(
        x_mic, x_ref, w_mic, b_mic, w_ref, b_ref, w_conv, b_conv
    )
    nc = _build_raw()
    from concourse.bass_utils import run_bass_kernel_spmd

    res = run_bass_kernel_spmd(nc, in_maps, core_ids=list(range(8)))
    out = np.zeros((B, C, T, F), dtype=np.float32)
    for (b, t0), r in zip(core_meta, res.results):
        o = np.asarray(r["out"], dtype=np.float32).reshape(TL, C, F)
        out[b, :, t0:t0 + TL, :] = o.transpose(1, 0, 2)
    return out


if __name__ == "__main__":
    z = np.load("/tmp/inputs.npz")
    ins = {k: z[k] for k in z.files}
    out = kernel(**ins)
    ref = np.load("/tmp/ref.npy")
    rel = np.abs(out - ref).max() / np.abs(ref).max()
    print("Relative error:", rel)
